# revision 22
# baseline (speedup 1.0000x reference)
"""Causal self-attention TRN2 Bass kernel (B=4, T=2048, C=1024, H=16, D=64).

Sharding: 8 cores = 4 batches x 2 head-groups (8 heads each). Each core computes
its batch's QKV for its heads, causal flash-style attention, and a partial
output projection; the host sums the two head-group partials per batch.

Design (~2x over the fp32r/DRAM-scratch baseline):
- All matmul operands bf16 (fp32 PSUM accumulation); host converts inputs.
- x, Q^T/K^T, V fully SBUF-resident -- no DRAM scratch round-trip.
- ACT engine is reserved for softmax exp during attention. QKV/V/proj bias
  staging runs on DVE (QK pr=0 on ACT at the idle head). The causal diag
  mask is a multiplicative 0/1 mask applied to p on the idle GPSIMD/Pool
  engine, off the S->exp critical path.
- Softmax denominators come free from a ones-augmented AV matmul; the
  reciprocal is exp(-ln(sums)) on ACT -- both functions forced into the one
  act-table set that holds them (see _gat_lnexp) so no table reloads.
- QK matmuls for head-pair pr+1 and projection matmuls are interleaved as
  PE fillers into attention steps (2 per key-block step) to cover the
  ACT-bound exp gaps; pr=3 walks q-chunks descending so proj unlocks early.

Per-core dataflow:
  lead-in: w_v+x DMA first; V = x Wv -> v_sb [t_kb, pr, j, (v|ones), d]
  interleaved per t-chunk with QK(pr=0) -> qk_sb [m, T] (dims on partitions).
  attention(pr): per q-chunk qc (512 q), per key block kb:
      [2 filler mms], S^T pair (row-tiled 2 heads, K=64, tile_position),
      exp (ACT, scale=1/8, bf16 out), Pool diag-mask, AV accumulate.
    Then per j: rec = exp(-ln(sums)) (ACT), oT = O^T * rec (DVE, bf16).
  proj: y^T = W_proj^T o^T partials, DVE bias staging -> yT DRAM (f32).
Host: y[b] = (yT[2b] + yT[2b+1]).T
"""

import numpy as np
from contextlib import ExitStack

import ml_dtypes

import concourse.bass as bass
import concourse.tile as tile
from concourse import bacc, mybir
import concourse.bass_interp as _bass_interp
import concourse.hw_specs as _hw_specs
import functools as _functools


@_functools.cache
def _gat_lnexp(module_arch):
    """Steer Exp/Ln onto the single set containing both, so the
    act-table fixpoint emits one LoadActFuncSet instead of thrashing."""
    orig = _hw_specs.get_activation_tables.__wrapped__(module_arch)
    out = {}
    for name, fns in orig.items():
        fns = set(fns)
        if name != "natural_log_exp_and_others":
            fns.discard(mybir.ActivationFunctionType.Exp)
            fns.discard(mybir.ActivationFunctionType.Ln)
        out[name] = fns
    return out


bacc.get_activation_tables = _gat_lnexp
_bass_interp.get_activation_tables = _gat_lnexp
from concourse.bass import ts
from concourse.bass_utils import run_bass_kernel_spmd

N_CORES = 8
B, T, C, H, D = 4, 2048, 1024, 16, 64
CB = C // 128          # 8 contraction blocks
NKB = T // 128         # 16 key blocks
NQC = T // 512         # 4 query chunks
NEG = -1.0e9

F32 = mybir.dt.float32
BF16 = mybir.dt.bfloat16
AF = mybir.ActivationFunctionType
OP = mybir.AluOpType

_CACHE = {}


def _build(phases=(1, 2, 3), reps=1, qk_dt=mybir.dt.bfloat16,
           p_dt=mybir.dt.bfloat16, ot_dt=mybir.dt.bfloat16,
           exact_recip=False, fillers=True, recip_lnexp=True,
           no_norm=False):
    nc = bacc.Bacc("TRN2", target_bir_lowering=False, debug=False, num_devices=N_CORES)

    xT = nc.dram_tensor("xT", [C, T], BF16, kind="ExternalInput").ap()
    w_qk = nc.dram_tensor("w_qk", [C, 1024], BF16, kind="ExternalInput").ap()
    w_v = nc.dram_tensor("w_v", [C, 512], BF16, kind="ExternalInput").ap()
    w_pr = nc.dram_tensor("w_pr", [512, C], BF16, kind="ExternalInput").ap()
    b_qk = nc.dram_tensor("b_qk", [1024], F32, kind="ExternalInput").ap()
    b_v = nc.dram_tensor("b_v", [128, 512], F32, kind="ExternalInput").ap()
    b_pr = nc.dram_tensor("b_pr", [C], F32, kind="ExternalInput").ap()
    yT = nc.dram_tensor("yT", [C, T], F32, kind="ExternalOutput").ap()

    xT_r = xT.rearrange("(cb p) t -> p cb t", p=128)
    w_qk_r = w_qk.rearrange("(cb p) m -> p cb m", p=128)
    w_v_r = w_v.rearrange("(cb p) m -> p cb m", p=128)
    w_pr_r = w_pr.rearrange("(pb p) m -> p pb m", p=128)
    b_qk_r = b_qk.rearrange("(m p) -> p m", p=128)
    b_pr_r = b_pr.rearrange("(m p) -> p m", p=128)
    yT_r = yT.rearrange("(m p) t -> p m t", p=128)

    with tile.TileContext(nc) as tc:
        with ExitStack() as ctx:
            # pools (SBUF KB/partition in comments)
            xp = ctx.enter_context(tc.tile_pool(name="xp", bufs=1))          # 32
            wqk_p = ctx.enter_context(tc.tile_pool(name="wqk", bufs=1))      # 16
            w2_p = ctx.enter_context(tc.tile_pool(name="w2", bufs=1))        # 8
            wpr_p = ctx.enter_context(tc.tile_pool(name="wpr", bufs=1))      # 8
            qk_p = ctx.enter_context(tc.tile_pool(name="qkp", bufs=1))       # 32
            v_p = ctx.enter_context(tc.tile_pool(name="vp", bufs=1))         # 32
            ot_p = ctx.enter_context(tc.tile_pool(name="ot", bufs=1))        # 16
            p_p = ctx.enter_context(tc.tile_pool(name="pp", bufs=4))         # 8
            rec_p = ctx.enter_context(tc.tile_pool(name="rec", bufs=4))      # 8
            st_p = ctx.enter_context(tc.tile_pool(name="st", bufs=4))        # 8
            misc = ctx.enter_context(tc.tile_pool(name="misc", bufs=1))      # ~2
            ps_s_p = ctx.enter_context(tc.tile_pool(name="ps_s", bufs=2, space="PSUM"))
            ps_f_p = ctx.enter_context(tc.tile_pool(name="ps_f", bufs=1, space="PSUM"))
            ps_o_p = ctx.enter_context(tc.tile_pool(name="ps_o", bufs=2, space="PSUM"))

            # constants
            b_qk_sb = misc.tile([128, 8], F32)
            nc.sync.dma_start(b_qk_sb[:], b_qk_r)
            b_v_sb = misc.tile([128, 512], F32)
            nc.sync.dma_start(b_v_sb[:], b_v)
            b_pr_sb = misc.tile([128, 8], F32)
            nc.sync.dma_start(b_pr_sb[:], b_pr_r)
            tri01 = misc.tile([128, 128], BF16)
            nc.gpsimd.memset(tri01[:], 1.0)
            # 1 where q(free) >= k(partition), 0 where q < k
            nc.gpsimd.affine_select(
                out=tri01[:], in_=tri01[:], compare_op=OP.is_ge, fill=0.0,
                base=0, pattern=[[1, 128]], channel_multiplier=-1,
            )

            # weights (w_v first so V matmuls can start early)
            w_v_sb = w2_p.tile([128, CB, 512], BF16)
            nc.sync.dma_start(w_v_sb[:], w_v_r)
            w_qk_sb = wqk_p.tile([128, CB, 1024], BF16)
            w_pr_sb = wpr_p.tile([128, 4, 1024], BF16)

            for _rep in range(reps):
                # x^T resident (bf16): [c-part, cb, t]
                x_sb = xp.tile([128, CB, T], BF16, tag="x", name=f"x_{_rep}")
                for tch in range(4):
                    nc.sync.dma_start(
                        x_sb[:, :, ts(tch, 512)], xT_r[:, :, ts(tch, 512)])
                if _rep == 0:
                    nc.sync.dma_start(w_qk_sb[:], w_qk_r)
                    nc.sync.dma_start(w_pr_sb[:], w_pr_r)

                # qk^T resident: [dim-part, m(4q+4k blocks), t]
                qk_sb = qk_p.tile([128, 8, T], qk_dt, tag="qk", name=f"qk_{_rep}")
                # v resident: [t-part(kb), kb, pr, j, (v|ones), d]
                v_sb = v_p.tile([128, NKB, 4, 2, 2, 64], BF16, tag="v",
                                name=f"v_{_rep}")
                nc.gpsimd.memset(v_sb[:, :, :, :, 1, :], 1.0)
                oT = ot_p.tile([128, 4, T], ot_dt, tag="oT", name=f"oT_{_rep}")

                filler = []
                pending_norm = []

                # ---------- V + QK0 + attention(pr0) interleaved ----------
                def v_unit(tch):
                    for vp in range(2):
                        ps = ps_s_p.tile([128, 2, 512], F32, tag="ps_s",
                                         name=f"v_{_rep}_{tch}_{vp}")
                        for cb in range(CB):
                            for h in (0, 1):
                                nc.tensor.matmul(
                                    ps[:, h],
                                    x_sb[:, cb, tch * 512 + (2 * vp + h) * 128:
                                         tch * 512 + (2 * vp + h + 1) * 128],
                                    w_v_sb[:, cb],
                                    start=(cb == 0), stop=(cb == CB - 1),
                                )
                        for h in (0, 1):
                            tb = tch * 4 + 2 * vp + h
                            # [128t, 512d] -> v_sb[t, tb, pr, j, 0, d]
                            nc.vector.tensor_tensor(
                                v_sb[:, tb, :, :, 0, :],
                                ps[:, h].rearrange("p (pr j d) -> p pr j d",
                                                   pr=4, j=2),
                                b_v_sb[:].rearrange("p (pr j d) -> p pr j d",
                                                    pr=4, j=2),
                                OP.add,
                            )

                def qk_unit(pr, tch, pool, on_act):
                    """QK matmuls for head-pair pr, t-chunk tch. Returns list of
                    closures: 16 matmuls then staging."""
                    ops = []
                    ps = [None]

                    def mk_mm(cb, h):
                        def f():
                            if ps[0] is None:
                                ps[0] = pool.tile(
                                    [128, 2, 512], F32, tag=pool.name,
                                    name=f"qk_{_rep}_{pr}_{tch}")
                            m = pr if h == 0 else 4 + pr
                            nc.tensor.matmul(
                                ps[0][:, h], w_qk_sb[:, cb, ts(m, 128)],
                                x_sb[:, cb, ts(tch, 512)],
                                start=(cb == 0), stop=(cb == CB - 1),
                            )
                        return f

                    for cb in range(CB):
                        for h in (0, 1):
                            ops.append(mk_mm(cb, h))

                    def mk_stage(h):
                        def f():
                            m = pr if h == 0 else 4 + pr
                            if on_act:
                                nc.scalar.activation(
                                    qk_sb[:, m, ts(tch, 512)], ps[0][:, h],
                                    AF.Identity, bias=b_qk_sb[:, m:m + 1])
                            else:
                                nc.vector.tensor_tensor(
                                    qk_sb[:, m, ts(tch, 512)], ps[0][:, h],
                                    b_qk_sb[:, m:m + 1].to_broadcast((128, 512)),
                                    OP.add)
                        return f

                    ops.append(mk_stage(0))
                    ops.append(mk_stage(1))
                    return ops

                def proj_unit(m, tch):
                    """Projection for output block m, t-chunk tch: 4 matmuls
                    then ACT staging + DMA out."""
                    ops = []
                    ps = [None]

                    def mk_mm(pb):
                        def f():
                            if ps[0] is None:
                                ps[0] = ps_f_p.tile(
                                    [128, 2, 512], F32, tag="ps_f",
                                    name=f"y_{_rep}_{m}_{tch}")
                            nc.tensor.matmul(
                                ps[0][:, 0], w_pr_sb[:, pb, ts(m, 128)],
                                oT[:, pb, ts(tch, 512)],
                                start=(pb == 0), stop=(pb == 3),
                            )
                        return f

                    for pb in range(4):
                        ops.append(mk_mm(pb))

                    def stage():
                        st = st_p.tile([128, 512], F32, tag="st",
                                       name=f"ys_{_rep}_{m}_{tch}")
                        nc.vector.tensor_tensor(
                            st[:], ps[0][:, 0],
                            b_pr_sb[:, m:m + 1].to_broadcast((128, 512)),
                            OP.add)
                        nc.sync.dma_start(yT_r[:, m, ts(tch, 512)], st[:])

                    ops.append(stage)
                    return ops



                def drain(n):
                    for _ in range(n):
                        if filler:
                            filler.pop(0)()

                def maybe_drain_all():
                    if not fillers:
                        drain(len(filler))

                def attn_qc(pr, qc, drate=2):
                    if True:
                        qTk = qk_sb[:, pr]        # [128, 2048] q dims on part
                        kTk = qk_sb[:, 4 + pr]    # [128, 2048] k dims on part
                        nkb = 4 * qc + 4
                        ps_o = [
                            ps_o_p.tile([128, 512], F32, tag="ps_o",
                                        name=f"o_{_rep}_{pr}_{qc}_{j}")
                            for j in (0, 1)
                        ]
                        ps_s = [None] * nkb
                        p_ts = [None] * nkb

                        def s_step(kb):
                            r = kb - 4 * qc
                            qlo = 128 * r if r > 0 else 0
                            s = ps_s_p.tile([128, 2, 512], F32, tag="ps_s",
                                            name=f"s_{_rep}_{pr}_{qc}_{kb}")
                            for j in (0, 1):
                                pb = j * 64
                                nc.tensor.matmul(
                                    s[:, j, qlo:512],
                                    kTk[pb:pb + 64, ts(kb, 128)],
                                    qTk[pb:pb + 64, qc * 512 + qlo:(qc + 1) * 512],
                                    start=True, stop=True, tile_position=(pb, 0),
                                )
                            p_t = p_p.tile([128, 2, 512], p_dt, tag="p",
                                           name=f"p_{_rep}_{pr}_{qc}_{kb}")
                            nc.scalar.activation(
                                p_t[:, :, qlo:512], s[:, :, qlo:512],
                                AF.Exp, scale=0.125,
                            )
                            if r >= 0:
                                nc.gpsimd.tensor_tensor(
                                    p_t[:, :, qlo:qlo + 128],
                                    p_t[:, :, qlo:qlo + 128],
                                    tri01[:, None, :].to_broadcast((128, 2, 128)),
                                    OP.mult,
                                )
                            ps_s[kb] = s
                            p_ts[kb] = p_t

                        def av_step(kb):
                            r = kb - 4 * qc
                            qlo = 128 * r if r > 0 else 0
                            for j in (0, 1):
                                nc.tensor.matmul(
                                    ps_o[j][:, qlo:512],
                                    v_sb[:, kb, pr, j],
                                    p_ts[kb][:, j, qlo:512],
                                    start=(kb == 0), stop=(kb == nkb - 1),
                                )

                        s_step(0)
                        for kb in range(nkb):
                            drain(drate)
                            if kb + 1 < nkb:
                                s_step(kb + 1)
                            av_step(kb)

                        def norm_unit(ps_o, pr, qc):
                          def f():
                            for j in (0, 1):
                              rec = rec_p.tile([128, 512], F32, tag="rec",
                                               name=f"rec_{_rep}_{pr}_{qc}_{j}")
                              if no_norm:
                                  nc.vector.tensor_copy(
                                      oT[j * 64:(j + 1) * 64, pr, ts(qc, 512)],
                                      ps_o[j][0:64, :])
                                  continue
                              if recip_lnexp:
                                  nc.scalar.activation(
                                      rec[64:128, :], ps_o[j][64:128, :], AF.Ln)
                                  nc.scalar.activation(
                                      rec[64:128, :], rec[64:128, :], AF.Exp,
                                      scale=-1.0)
                              else:
                                  nc.vector.reciprocal(
                                      rec[64:128, :], ps_o[j][64:128, :])
                              nc.vector.tensor_tensor(
                                  oT[j * 64:(j + 1) * 64, pr, ts(qc, 512)],
                                  ps_o[j][0:64, :], rec[64:128, :], OP.mult,
                              )
                          return f
                        norm_unit(ps_o, pr, qc)()

                for tch in range(4 if 1 in phases else 0):
                    v_unit(tch)
                    if 2 in phases:
                        for f in qk_unit(0, tch, ps_s_p, on_act=True):
                            f()
                        filler.extend(qk_unit(1, tch, ps_f_p, on_act=False))
                        maybe_drain_all()
                        attn_qc(0, tch)

                # pr=1..3 with fillers.
                for pr in range(1, 4) if 2 in phases else []:
                    if pr < 3:
                        for tch in range(4):
                            filler.extend(
                                qk_unit(pr + 1, tch, ps_f_p, on_act=False))
                        maybe_drain_all()
                    qcs = [3, 2, 1, 0] if pr == 3 else list(range(NQC))
                    for qi, qc in enumerate(qcs):
                        if pr == 3 and qi > 0 and 3 in phases:
                            for m in range(8):
                                filler.extend(proj_unit(m, qcs[qi - 1]))
                            maybe_drain_all()
                        attn_qc(pr, qc, drate=5 if pr == 3 else 2)
                # drain leftover fillers (incl. proj for last processed qc)
                if 3 in phases and 2 in phases:
                    for m in range(8):
                        filler.extend(proj_unit(m, 0))
                drain(len(filler))

                if 2 not in phases and 8 not in phases and 9 not in phases \
                        and 10 not in phases:
                    # keep yT written so the sim has an output dep
                    st0 = st_p.tile([128, 512], F32, tag="st", name="dummy_out")
                    nc.gpsimd.memset(st0[:], 0.0)
                    nc.sync.dma_start(yT_r[:, 0, 0:512], st0[:])

                def dump(dst_m, dst_tch, src_ap):
                    st = st_p.tile([128, 512], F32, tag="st",
                                   name=f"dump_{dst_m}_{dst_tch}")
                    nc.vector.tensor_copy(st[:], src_ap)
                    nc.sync.dma_start(yT_r[:, dst_m, ts(dst_tch, 512)], st[:])

                if 8 in phases:      # dump x_sb -> yT rows (cb-major)
                    for cb in range(8):
                        for tch in range(4):
                            dump(cb, tch, x_sb[:, cb, ts(tch, 512)])
                def dump4(dst_m, dst_tch, src_ap):
                    st = st_p.tile([128, 512], F32, tag="st",
                                   name=f"dump4_{dst_m}_{dst_tch}")
                    nc.vector.tensor_copy(
                        st[:].rearrange("p (pr j d) -> p pr j d", pr=4, j=2),
                        src_ap)
                    nc.sync.dma_start(yT_r[:, dst_m, ts(dst_tch, 512)], st[:])

                if 9 in phases:      # dump v_sb data cols -> yT
                    for tb in range(16):
                        dump4(tb // 4, tb % 4, v_sb[:, tb, :, :, 0, :])
                    for tb in range(8):  # ones planes for tb 0..7
                        dump4(4 + tb // 4, tb % 4, v_sb[:, tb, :, :, 1, :])
                if 10 in phases:     # dump qk_sb -> yT
                    for m in range(8):
                        for tch in range(4):
                            dump(m, tch, qk_sb[:, m, ts(tch, 512)])

    nc.compile()
    return nc


def _in_maps(x, W_attn, b_attn, W_proj, b_proj):
    bf = ml_dtypes.bfloat16
    maps = []
    for b in range(B):
        for g in range(2):
            cs = slice(g * 512, (g + 1) * 512)
            maps.append({
                "xT": np.ascontiguousarray(x[b].T.astype(bf)),
                "w_qk": np.ascontiguousarray(np.concatenate(
                    [W_attn[:, cs], W_attn[:, 1024 + cs.start:1024 + cs.stop]],
                    axis=1).astype(bf)),
                "w_v": np.ascontiguousarray(
                    W_attn[:, 2048 + cs.start:2048 + cs.stop].astype(bf)),
                "w_pr": np.ascontiguousarray(W_proj[cs, :].astype(bf)),
                "b_qk": np.ascontiguousarray(np.concatenate(
                    [b_attn[cs], b_attn[1024 + cs.start:1024 + cs.stop]])),
                "b_v": np.ascontiguousarray(np.tile(
                    b_attn[2048 + cs.start:2048 + cs.stop][None, :], (128, 1))),
                "b_pr": np.ascontiguousarray(b_proj),
            })
    return maps


def kernel(x, W_attn, b_attn, W_proj, b_proj):
    x = np.asarray(x, dtype=np.float32)
    W_attn = np.asarray(W_attn, dtype=np.float32)
    b_attn = np.asarray(b_attn, dtype=np.float32)
    W_proj = np.asarray(W_proj, dtype=np.float32)
    b_proj = np.asarray(b_proj, dtype=np.float32)

    if "nc" not in _CACHE:
        _CACHE["nc"] = _build()
    nc = _CACHE["nc"]

    maps = _in_maps(x, W_attn, b_attn, W_proj, b_proj)
    last_exc = None
    for attempt in range(3):
        try:
            res = run_bass_kernel_spmd(nc, maps, core_ids=list(range(N_CORES)))
            break
        except Exception as exc:  # transient device wedges recover on retry
            last_exc = exc
            if attempt == 2:
                raise
            import time as _time
            _time.sleep(5)
    y = np.empty((B, T, C), dtype=np.float32)
    for b in range(B):
        y[b] = (res.results[2 * b]["yT"] + res.results[2 * b + 1]["yT"]).T
    return y
